# revision 5
# baseline (speedup 1.0000x reference)
"""DelayGNNStage Trainium2 kernel: 3-layer, 2-hop message-passing GNN.

Strategy (graph/data parallel over 8 NeuronCores):
  - Nodes partitioned across cores by destination (12800 padded rows each,
    25 superblocks x 512 dst rows).
  - Gather tables (node features) are bf16, partition-major, AllGathered to
    every core's HBM between layers.
  - Per (layer, hop, superblock): edges are bucketed by source table row
    range (4 buckets of 25600 rows so relative indices fit int16) and
    fetched with large `dma_gather` calls (one per bucket, up to 2048 rows
    each) instead of per-tile indirect DMAs.
  - Segment-sum via TensorE: per 128-edge tile, a bf16 one-hot [edge, 512]
    matrix (is_equal vs iota) maps gathered rows to dst slots; matmuls
    accumulate aggT[d, dst] in PSUM over the whole superblock.
  - aggT @ (softmax(alpha)-scaled W) for both hops accumulates in PSUM,
    then relu + residual on the f32 SBUF-resident x slice.
"""

import time

import numpy as np
import ml_dtypes

import concourse.bass as bass
import concourse.mybir as mybir
import concourse.mybir as mb
from concourse.tile import TileContext
from concourse import library_config

# problem constants (hardcoded per contract)
N, E, D, T, K, NU = 100000, 1600000, 128, 3, 2, 1
NCORES = 8
NPC = 12800          # padded nodes per core (25 superblocks x 512)
NSB = NPC // 512     # superblocks per core
NPAD = NCORES * NPC  # 102400
NROWS = NPAD         # gather-table rows (row = q*100+g)
BUCKET = 25600       # src row range per bucket (fits int16)
NBUCK = NROWS // BUCKET  # 4
MAXT = 16            # max 128-row tiles per dma_gather


def _split_multiwaits(nc):
    """Walrus in this container only accepts one sem-wait per instruction;
    hoist extras onto same-engine NoOps immediately before."""
    for fn in nc.m.functions:
        for bb in fn.blocks:
            newinsts = []
            for ins in bb.instructions:
                si = ins.sync_info
                try:
                    waits = list(si.on_wait) if si is not None else []
                except Exception:
                    waits = []
                if len(waits) > 1:
                    keep = waits[-1]
                    for w in waits[:-1]:
                        nop = mb.InstNoOp(
                            name=nc.get_next_instruction_name(), ins=[], outs=[])
                        nop.engine = ins.engine
                        nop.sync_info = mb.SyncInfo(on_wait=[w], on_update=[])
                        newinsts.append(nop)
                    ins.sync_info = mb.SyncInfo(
                        on_wait=[keep], on_update=list(si.on_update))
                newinsts.append(ins)
            bb.instructions = newinsts


def _preprocess(x, edge_index, edge_attr, W, b, alpha):
    """Host-side sharding/scheduling. Returns per-core input maps and the
    common (cross-core) chunk schedule."""
    x = np.asarray(x, dtype=np.float32)
    ei = np.asarray(edge_index)
    ea = np.asarray(edge_attr)
    W = np.asarray(W, dtype=np.float32)
    b = np.asarray(b, dtype=np.float32)
    alpha = np.asarray(alpha, dtype=np.float32)

    # softmax over k, fold into W; fold bias rows
    aexp = np.exp(alpha - alpha.max(axis=1, keepdims=True))
    a = aexp / aexp.sum(axis=1, keepdims=True)          # [T, K]
    Ws = W * a[:, :, None, None]                        # [T, K, D, D]
    bias_rows = (a[:, :, None] * b).sum(axis=1)         # [T, D]
    has_bias = bool(np.abs(bias_rows).max() > 0)

    # node padding: original v -> core v//12500, padded id c*NPC + (v - c*12500)
    orig_per_core = N // NCORES  # 12500
    v = np.arange(N)
    core_of = v // orig_per_core
    pad_id = core_of * NPC + (v - core_of * orig_per_core)

    x_pad = np.zeros((NPAD, D), dtype=np.float32)
    x_pad[pad_id] = x
    # partition-major packing: X_pm[c*128+p, g*128+d] = x_pad[c*12800+g*128+p, d]
    x_pm = (x_pad.reshape(NCORES, NPC // 128, 128, D)
            .transpose(0, 2, 1, 3).reshape(NCORES * 128, NPC // 128 * D))

    src_p = pad_id[ei[0]]
    dst_p = pad_id[ei[1]]
    # src table row: node c*12800+g*128+p -> row (c*128+p)*100 + g
    sc_ = src_p // NPC
    rem = src_p - sc_ * NPC
    g_ = rem // 128
    p_ = rem - g_ * 128
    src_row_all = ((sc_ * 128 + p_) * (NPC // 128) + g_).astype(np.int64)

    hops = []
    for k in (1, 2):
        sel = ea == k
        r_k = src_row_all[sel]
        d_k = dst_p[sel]
        dcore = d_k // NPC
        dloc = d_k - dcore * NPC
        sb = dloc // 512
        dl = dloc - sb * 512
        bk = r_k // BUCKET

        # per core: sorted edge lists + counts per (sb, bucket)
        per_core = []
        cnts = np.zeros((NCORES, NSB, NBUCK), dtype=np.int64)
        for c in range(NCORES):
            m = dcore == c
            rc, sc, dc, bc = r_k[m], sb[m], dl[m], bk[m]
            order = np.lexsort((rc, bc, sc))
            rc, sc, dc, bc = rc[order], sc[order], dc[order], bc[order]
            key = sc * NBUCK + bc
            cnts[c] = np.bincount(key, minlength=NSB * NBUCK).reshape(NSB, NBUCK)
            per_core.append((rc, sc, dc, bc, key))

        # common tile counts per (sb, bucket), split into chunks of <= MAXT
        ntiles = np.ceil(cnts.max(axis=0) / 128).astype(np.int64)  # [NSB, NBUCK]
        chunks = []          # per sb: list of (bucket, nt)
        for s in range(NSB):
            ch = []
            for bq in range(NBUCK):
                nt = int(ntiles[s, bq])
                while nt > 0:
                    take = min(nt, MAXT)
                    ch.append((bq, take))
                    nt -= take
            assert ch, f"superblock {s} has no edges"
            chunks.append(ch)
        total_tiles = int(ntiles.sum())
        # tile offset of each (sb, bucket) in emission order
        off_sb_bk = np.zeros((NSB, NBUCK), dtype=np.int64)
        acc = 0
        for s in range(NSB):
            for bq in range(NBUCK):
                off_sb_bk[s, bq] = acc
                acc += int(ntiles[s, bq])

        idx16 = []
        relv = []
        for c in range(NCORES):
            rc, sc, dc, bc, key = per_core[c]
            # rank of each edge within its (sb, bucket) group
            starts = np.zeros(NSB * NBUCK + 1, dtype=np.int64)
            np.cumsum(np.bincount(key, minlength=NSB * NBUCK), out=starts[1:])
            rank = np.arange(len(rc)) - starts[key]
            slot = off_sb_bk[sc, bc] * 128 + rank
            idx_flat = np.zeros(total_tiles * 128, dtype=np.int16)
            rel_flat = np.full(total_tiles * 128, -1.0, dtype=np.float32)
            idx_flat[slot] = (rc - bc * BUCKET).astype(np.int16)
            rel_flat[slot] = dc
            # per-chunk int16 packing: block [16, nt*8] (F-order 16-wrap)
            blocks = []
            pos = 0
            for s in range(NSB):
                for (bq, nt) in chunks[s]:
                    seg = idx_flat[pos * 128:(pos + nt) * 128]
                    blocks.append(seg.reshape(-1, 16).T)
                    pos += nt
            packed = np.concatenate(blocks, axis=1)        # [16, total*8]
            idx16.append(np.tile(packed, (8, 1)))          # [128, total*8]
            relv.append(rel_flat.reshape(total_tiles, 128).T.copy())
        hops.append(dict(chunks=chunks, total_tiles=total_tiles,
                         idx16=idx16, rel=relv))

    return dict(x_pad=x_pad, x_pm=x_pm, pad_id=pad_id, Ws=Ws,
                bias_rows=bias_rows, has_bias=has_bias, hops=hops)


def _build_kernel(meta, split=True):
    """Emit the bass kernel for the common schedule in `meta`."""
    hops = meta["hops"]
    has_bias = meta["has_bias"]
    T1 = hops[0]["total_tiles"]
    T2 = hops[1]["total_tiles"]

    nc = bass.Bass(num_devices=NCORES, num_swdge_queues=4)
    f32 = mybir.dt.float32
    bf16 = mybir.dt.bfloat16
    i16 = mybir.dt.int16

    # partition-major table layout: row q=(c*128+p), col (g*128+d) holds
    # node c*12800+g*128+p feature d; gather view row = q*100+g
    x0s = nc.dram_tensor("x0s", [128, NPC // 128 * D], f32,
                         kind="ExternalInput")
    cc_x0 = nc.dram_tensor("cc_x0", [128, NPC // 128 * D], bf16)
    X0 = nc.dram_tensor("X0i", [NCORES * 128, NPC // 128 * D], bf16,
                        addr_space="Shared")
    idx1 = nc.dram_tensor("idx1", [128, T1 * 8], i16, kind="ExternalInput")
    idx2 = nc.dram_tensor("idx2", [128, T2 * 8], i16, kind="ExternalInput")
    rel1 = nc.dram_tensor("rel1", [128, T1], f32, kind="ExternalInput")
    rel2 = nc.dram_tensor("rel2", [128, T2], f32, kind="ExternalInput")
    Wd = nc.dram_tensor("Wd", [T * K, D, D], bf16, kind="ExternalInput")
    IOTA = nc.dram_tensor("IOTA", [128, 512], f32, kind="ExternalInput")
    BIASD = nc.dram_tensor("BIASD", [T, 128, D], f32, kind="ExternalInput")
    Y = nc.dram_tensor("Y", [128, NPC // 128 * D], f32, kind="ExternalOutput")

    cc_in = [nc.dram_tensor(f"cc_in{t}", [128, NPC // 128 * D], bf16)
             for t in range(T - 1)]
    cc_out = [nc.dram_tensor(f"cc_out{t}", [NCORES * 128, NPC // 128 * D],
                             bf16, addr_space="Shared") for t in range(T - 1)]

    # gather tables per (t, k): k=1 reads xs[t], k=2 reads xs[t-1] (t=0,1 -> x0)
    tables = {(0, 1): X0, (0, 2): X0}
    if T > 1:
        tables.update({(1, 1): cc_out[0], (1, 2): X0})
    if T > 2:
        tables.update({(2, 1): cc_out[1], (2, 2): cc_out[0]})

    idx_d = {1: idx1, 2: idx2}
    rel_d = {1: rel1, 2: rel2}
    Tk_d = {1: T1, 2: T2}

    # tile offsets per (k, sb)
    tile_off = {}
    for ki, k in enumerate((1, 2)):
        off = 0
        for s in range(NSB):
            tile_off[(k, s)] = off
            off += sum(nt for _, nt in hops[ki]["chunks"][s])

    qrot = [0]

    with TileContext(nc) as tc:
        nc.gpsimd.load_library(library_config.mlp)
        nidx_regs = {}

        def nidx_reg(v):
            if v not in nidx_regs:
                nidx_regs[v] = nc.gpsimd.to_reg(v)
            return nidx_regs[v]
        with (
            tc.tile_pool(name="const", bufs=1) as cpool,
            tc.tile_pool(name="blob", bufs=1) as bpool,
            tc.tile_pool(name="xres", bufs=1) as xpool,
            tc.tile_pool(name="agg2res", bufs=1) as a2pool,
            tc.tile_pool(name="mbuf", bufs=3) as mpool,
            tc.tile_pool(name="sbuf_s", bufs=3) as spool,
            tc.tile_pool(name="agg1", bufs=2) as a1pool,
            tc.tile_pool(name="relu", bufs=3) as rpool,
            tc.tile_pool(name="psumA", bufs=2, space="PSUM") as ppoolA,
            tc.tile_pool(name="psumB", bufs=2, space="PSUM") as ppoolB,
            tc.tile_pool(name="psumO", bufs=2, space="PSUM") as ppoolO,
        ):
            iota_sb = cpool.tile([128, 512], f32, name="iota_sb")
            nc.sync.dma_start(out=iota_sb[:], in_=IOTA[:])
            w_sb = cpool.tile([128, T * K * D], bf16, name="w_sb")
            nc.sync.dma_start(
                out=w_sb[:].rearrange("p (g d) -> p g d", d=D),
                in_=Wd[:].rearrange("g p d -> p g d"))
            if has_bias:
                bias_sb = cpool.tile([128, T * D], f32, name="bias_sb")
                nc.sync.dma_start(
                    out=bias_sb[:].rearrange("p (t d) -> p t d", d=D),
                    in_=BIASD[:].rearrange("t p d -> p t d"))

            idx_sb = {}
            rel_sb = {}
            for k in (1, 2):
                it = bpool.tile([128, Tk_d[k] * 8], i16, name=f"idx_sb{k}")
                nc.sync.dma_start(out=it[:], in_=idx_d[k][:])
                rt_ = bpool.tile([128, Tk_d[k]], f32, name=f"rel_sb{k}")
                nc.sync.dma_start(out=rt_[:], in_=rel_d[k][:])
                idx_sb[k] = it
                rel_sb[k] = rt_

            x_sl = xpool.tile([128, NPC], f32, name="x_sl")  # [p, blk*128+d]
            nc.sync.dma_start(out=x_sl[:], in_=x0s[:])
            # bf16 copy of the local slice -> collective -> full bf16 table
            nc.gpsimd.dma_start(out=cc_x0[:], in_=x_sl[:])
            nc.gpsimd.collective_compute(
                "AllGather", mybir.AluOpType.bypass,
                replica_groups=[list(range(NCORES))],
                ins=[cc_x0[:]], outs=[X0[:]])

            agg2 = a2pool.tile([128, NSB * 512], bf16, name="agg2")

            def hop_aggregate(t, k, s, ppool, ptag):
                """Gathers + segment matmuls for (layer t, hop k, superblock
                s). Returns the PSUM aggT tile [128, 512] (f32)."""
                ki = k - 1
                ch = hops[ki]["chunks"][s]
                base_tile = tile_off[(k, s)]
                nt_total = sum(nt for _, nt in ch)
                psum = ppool.tile([128, 512], f32, space="PSUM",
                                  name=f"ps{t}_{k}_{s}", tag=ptag)
                table = tables[(t, k)][:].rearrange("q (g d) -> (q g) d", d=D)
                jglob = 0
                toff = base_tile
                for (bq, nt) in ch:
                    m = mpool.tile([128, MAXT * 128], bf16,
                                   name=f"m{t}_{k}_{s}_{toff}", tag="m")
                    nc.gpsimd.dma_gather(
                        m[:, :nt * 128].rearrange("p (j d) -> p j d", d=D),
                        table[bq * BUCKET:, :],
                        idx_sb[k][:, toff * 8:(toff + nt) * 8],
                        nt * 128, nidx_reg(nt * 128), D,
                        queue_num=qrot[0], single_packet=False,
                    )
                    qrot[0] = (qrot[0] + 1) % 4
                    sm = spool.tile([128, MAXT * 512], bf16,
                                    name=f"s{t}_{k}_{s}_{toff}", tag="s")
                    nc.vector.tensor_tensor(
                        out=sm[:, :nt * 512].rearrange("p (j c) -> p j c", c=512),
                        in0=rel_sb[k][:, toff:toff + nt]
                            .to_broadcast([128, nt, 512]),
                        in1=iota_sb[:].rearrange("p (j c) -> p j c", j=1)
                            .to_broadcast([128, nt, 512]),
                        op=mybir.AluOpType.is_equal,
                    )
                    for j in range(nt):
                        nc.tensor.matmul(
                            out=psum[:],
                            lhsT=m[:, j * 128:(j + 1) * 128],
                            rhs=sm[:, j * 512:(j + 1) * 512],
                            start=(jglob == 0), stop=(jglob == nt_total - 1),
                            skip_group_check=True,
                        )
                        jglob += 1
                    toff += nt
                return psum

            for t in range(T):
                # phase A: hop 2 (older table) -> resident agg2 (bf16)
                for s in range(NSB):
                    psA = hop_aggregate(t, 2, s, ppoolA, "psA")
                    nc.scalar.copy(out=agg2[:, s * 512:(s + 1) * 512],
                                   in_=psA[:])
                # phase B: hop 1 (fresh table) + combine + residual + store
                for s in range(NSB):
                    psB = hop_aggregate(t, 1, s, ppoolB, "psB")
                    a1 = a1pool.tile([128, 512], bf16, name=f"a1_{t}_{s}",
                                     tag="a1")
                    nc.scalar.copy(out=a1[:], in_=psB[:])
                    op = ppoolO.tile([128, 512], f32, space="PSUM",
                                     name=f"op{t}_{s}", tag="op")
                    for g in range(4):
                        sl = slice(g * 128, (g + 1) * 128)
                        nc.tensor.matmul(
                            out=op[:, sl], lhsT=a1[:, sl],
                            rhs=w_sb[:, (t * K + 0) * D:(t * K + 1) * D],
                            start=True, stop=False, skip_group_check=True)
                        nc.tensor.matmul(
                            out=op[:, sl],
                            lhsT=agg2[:, s * 512 + g * 128:
                                      s * 512 + (g + 1) * 128],
                            rhs=w_sb[:, (t * K + 1) * D:(t * K + 2) * D],
                            start=False, stop=True, skip_group_check=True)
                    rt = rpool.tile([128, 512], f32, name=f"rt{t}_{s}", tag="rt")
                    if has_bias:
                        nc.vector.tensor_tensor(
                            out=rt[:],
                            in0=op[:].rearrange("p (g d) -> p g d", d=D),
                            in1=bias_sb[:, t * D:(t + 1) * D]
                                .rearrange("p (o d) -> p o d", o=1)
                                .to_broadcast([128, 4, D]),
                            op=mybir.AluOpType.add)
                        nc.scalar.activation(
                            out=rt[:], in_=rt[:],
                            func=mybir.ActivationFunctionType.Relu)
                    else:
                        nc.scalar.activation(
                            out=rt[:], in_=op[:],
                            func=mybir.ActivationFunctionType.Relu)
                    col = s * 512
                    nc.vector.tensor_add(
                        out=x_sl[:, col:col + 512],
                        in0=x_sl[:, col:col + 512], in1=rt[:])
                if t < T - 1:
                    nc.gpsimd.dma_start(out=cc_in[t][:], in_=x_sl[:])
                    nc.gpsimd.collective_compute(
                        "AllGather",
                        mybir.AluOpType.bypass,
                        replica_groups=[list(range(NCORES))],
                        ins=[cc_in[t][:]],
                        outs=[cc_out[t][:]],
                    )
                else:
                    nc.sync.dma_start(out=Y[:], in_=x_sl[:])
    # spread plain SWDGE DMAs across the 4 SWDGE queues
    qi = 0
    for fn in nc.m.functions:
        for bb in fn.blocks:
            for ins in bb.instructions:
                if (type(ins).__name__ == "InstDMACopy"
                        and getattr(ins, "queue", "") == "qPoolDynamic"
                        and ins.engine == mybir.EngineType.Pool):
                    ins.queue = f"qPoolDynamic{qi or ''}"
                    qi = (qi + 1) % 4
    mybir.codegen_inst_isa_subclasses(nc)
    if split:
        _split_multiwaits(nc)
    return nc


class _PjrtRunner:
    """Jitted PJRT runner (mimics bass2jax.run_bass_via_pjrt) kept alive so
    repeated executions reuse the compiled NEFF."""

    def __init__(self, nc, n_cores):
        import jax
        from jax.sharding import Mesh, PartitionSpec
        from jax.experimental.shard_map import shard_map
        from concourse.bass2jax import (
            _bass_exec_p, install_neuronx_cc_hook, partition_id_tensor)

        install_neuronx_cc_hook()
        self.jax = jax
        self.n_cores = n_cores
        pname = nc.partition_id_tensor.name if nc.partition_id_tensor else None

        in_names, out_names, out_avals, zero_outs = [], [], [], []
        for alloc in nc.m.functions[0].allocations:
            if not isinstance(alloc, mybir.MemoryLocationSet):
                continue
            name = alloc.memorylocations[0].name
            if alloc.kind == "ExternalInput":
                if name != pname:
                    in_names.append(name)
            elif alloc.kind == "ExternalOutput":
                out_names.append(name)
                shape = tuple(alloc.tensor_shape)
                dtype = mybir.dt.np(alloc.dtype)
                out_avals.append(jax.core.ShapedArray(shape, dtype))
                zero_outs.append(np.zeros(shape, dtype))
        self.in_names = list(in_names)
        self.out_names = out_names
        self.out_avals = out_avals
        self.zero_outs = zero_outs
        n_params = len(in_names)
        all_names = in_names + out_names
        if pname is not None:
            all_names.append(pname)

        def _body(*args):
            operands = list(args)
            if pname is not None:
                operands.append(partition_id_tensor())
            outs = _bass_exec_p.bind(
                *operands,
                out_avals=tuple(out_avals),
                in_names=tuple(all_names),
                out_names=tuple(out_names),
                lowering_input_output_aliases=(),
                sim_require_finite=True,
                sim_require_nnan=True,
                nc=nc,
            )
            return tuple(outs)

        devices = jax.devices()[:n_cores]
        self.mesh = Mesh(np.asarray(devices), ("core",))
        in_specs = (PartitionSpec("core"),) * (n_params + len(out_names))
        out_specs = (PartitionSpec("core"),) * len(out_names)
        self.sharded = jax.jit(
            shard_map(_body, mesh=self.mesh, in_specs=in_specs,
                      out_specs=out_specs, check_rep=False),
            keep_unused=True,
        )

    def upload(self, in_maps):
        from jax.sharding import NamedSharding, PartitionSpec
        sh = NamedSharding(self.mesh, PartitionSpec("core"))
        args = []
        for name in self.in_names:
            cat = np.concatenate([np.asarray(m[name]) for m in in_maps], axis=0)
            args.append(self.jax.device_put(cat, sh))
        for z in self.zero_outs:
            cat = np.zeros((self.n_cores * z.shape[0], *z.shape[1:]), z.dtype)
            args.append(self.jax.device_put(cat, sh))
        return args

    def run(self, args):
        outs = self.sharded(*args)
        self.jax.block_until_ready(outs)
        return outs

    def timed_run(self, args, iters=3):
        outs = self.run(args)
        times = []
        for _ in range(iters):
            t0 = time.perf_counter()
            outs = self.run(args)
            times.append(time.perf_counter() - t0)
        return outs, min(times)

    def results(self, outs):
        res = []
        for c in range(self.n_cores):
            d = {}
            for i, name in enumerate(self.out_names):
                full = np.asarray(outs[i])
                per = full.reshape(self.n_cores, *self.out_avals[i].shape)
                d[name] = per[c]
            res.append(d)
        return res


_LAST_RUNNER = None
_BUILD_CACHE = {}


def kernel(x, edge_index, edge_attr, W, b, alpha):
    global _LAST_RUNNER
    import hashlib
    ekey = hashlib.sha1(np.ascontiguousarray(edge_index)).hexdigest() + \
        hashlib.sha1(np.ascontiguousarray(edge_attr)).hexdigest()
    if ekey in _BUILD_CACHE:
        meta_sched, nc, runner = _BUILD_CACHE[ekey]
        meta = _preprocess(x, edge_index, edge_attr, W, b, alpha)
        meta["hops"] = meta_sched  # identical; keep cached object
    else:
        meta = _preprocess(x, edge_index, edge_attr, W, b, alpha)
        nc = _build_kernel(meta)
        runner = None
    hops = meta["hops"]

    iota = np.tile(np.arange(512, dtype=np.float32)[None, :], (128, 1))
    Wflat = meta["Ws"].reshape(T * K, D, D).astype(ml_dtypes.bfloat16)
    biasd = np.tile(meta["bias_rows"][:, None, :], (1, 128, 1)).astype(np.float32)

    in_maps = []
    for c in range(NCORES):
        in_maps.append({
            "x0s": meta["x_pm"][c * 128:(c + 1) * 128],
            "idx1": hops[0]["idx16"][c],
            "idx2": hops[1]["idx16"][c],
            "rel1": hops[0]["rel"][c],
            "rel2": hops[1]["rel"][c],
            "Wd": Wflat,
            "IOTA": iota,
            "BIASD": biasd,
        })

    if runner is None:
        runner = _PjrtRunner(nc, NCORES)
        _BUILD_CACHE[ekey] = (hops, nc, runner)
    args = runner.upload(in_maps)
    outs = runner.run(args)
    results = runner.results(outs)
    _LAST_RUNNER = (runner, args)

    # Y is partition-major [128, (g d)] per core -> unpack to [NPAD, D]
    out_pm = np.stack([results[c]["Y"] for c in range(NCORES)], axis=0)
    out_pad = (out_pm.reshape(NCORES, 128, NPC // 128, D)
               .transpose(0, 2, 1, 3).reshape(NPAD, D))
    return out_pad[meta["pad_id"]]
